# revision 23
# baseline (speedup 1.0000x reference)
"""DiT block with MoE (top-2 of 8 experts) on 8 Trainium2 NeuronCores.

Strategy:
  Phase 1 (attention branch): data-parallel over batch (core b <- batch b).
    Host precomputes adaLN modulation + LN (elementwise) and feeds the
    modulated input transposed [D, S]. Device does qkv projection,
    attention (softmax without max-subtraction: logits are bounded ~12),
    and the output projection. All activations live in [feature, token]
    layout so every matmul contracts over the partition dim with weights
    as the stationary operand in natural layout.
  Host: residual + gate, LN, gate logits, top-2 routing, token dispatch.
  Phase 2 (MoE experts): expert-parallel. Each expert's MLP is split in
    half along MLP_H; each core processes one half of a "big" expert and
    one half of a "small" expert (sorted pairing) for load balance.
    Tokens arrive gathered+padded per expert, transposed [D, S].
  Host: weighted scatter-add combine + final residual.

Matmuls run as float32r (full PE rate at moving-dim >= 256).
"""

import os
from contextlib import ExitStack

import numpy as np

import concourse.bass as bass
import concourse.mybir as mybir
import concourse.tile as tile
from concourse import bacc
from concourse.bass import ds, ts
from concourse.bass_utils import run_bass_kernel_spmd

P = 128
D = 1152
S = 256  # tokens per batch
B = 8
H = 16
HD = 72  # head dim
E = 8
MLP_H = 4608
HH = MLP_H // 2  # half expert width
ND = D // P  # 9
NH = HH // P  # 18
EPS = 1e-6
F32 = mybir.dt.float32
F32R = mybir.dt.float32r
# KSIM_TANH=1 swaps the gelu LUT for tanh (CoreSim doesn't model the gelu
# LUT); dev harnesses match their numpy reference accordingly.
GELU = (
    mybir.ActivationFunctionType.Tanh
    if os.environ.get("KSIM_TANH")
    else mybir.ActivationFunctionType.Gelu_apprx_tanh
)
EXP = mybir.ActivationFunctionType.Exp
IDENT = mybir.ActivationFunctionType.Identity

_prog_cache = {}
_ONES = np.ones((128, 72), dtype=np.float32)


TRACE = False
LAST_EXEC_NS = []
LAST_PROFILES = []


def _run(nc, in_maps):
    res = run_bass_kernel_spmd(
        nc, in_maps, list(range(len(in_maps))), trace=TRACE
    )
    if TRACE:
        LAST_EXEC_NS.append(res.exec_time_ns)
        LAST_PROFILES.append(res)
    return res.results


def _new_nc():
    return bacc.Bacc(
        "TRN2", target_bir_lowering=False, debug=False, num_devices=8
    )


# --------------------------------------------------------------------------
# Phase 1: attention branch, one batch element per core.
# in:  hT [D, S]   modulated+LN'd input, transposed
#      qkv_w [D, 3D], qkv_b [3D], proj_w [D, D]
# out: attn [S, D] = softmax((h qkv_w)_q (h qkv_w)_k^T / sqrt(hd)) v proj_w
#      (v-bias and proj bias are folded in on the host: with row-stochastic
#       attention, o = att@(v + 1 b_v^T) = att@v + 1 b_v^T, so those biases
#       become a constant vector added after proj.)
# --------------------------------------------------------------------------
def _build_phase1():
    nc = _new_nc()
    hT_d = nc.dram_tensor("hT", [D, S], F32R, kind="ExternalInput").ap()
    qkvw_d = nc.dram_tensor("qkv_w", [D, 3 * D], F32R, kind="ExternalInput").ap()
    qkvb_d = nc.dram_tensor("qkv_b", [3 * D], F32, kind="ExternalInput").ap()
    projw_d = nc.dram_tensor("proj_w", [D, D], F32R, kind="ExternalInput").ap()
    ones_d = nc.dram_tensor("ones", [P, HD], F32R, kind="ExternalInput").ap()
    attn_d = nc.dram_tensor("attn", [S, D], F32, kind="ExternalOutput").ap()

    scale = float(HD) ** -0.5

    with tile.TileContext(nc) as tc, ExitStack() as ctx:
        sb = ctx.enter_context(tc.tile_pool(name="sb", bufs=1))
        wvp = ctx.enter_context(tc.tile_pool(name="wvp", bufs=2))
        wgp = ctx.enter_context(tc.tile_pool(name="wgp", bufs=4))
        pwp = ctx.enter_context(tc.tile_pool(name="pwp", bufs=6))
        small = ctx.enter_context(tc.tile_pool(name="small", bufs=4))

        hTt = sb.tile([P, ND, S], F32R, tag="hT")
        nc.sync.dma_start(hTt[:], hT_d.rearrange("(o p) s -> p o s", p=P))

        qkb = sb.tile([HD, 2 * H], F32, tag="qkb")  # q,k biases per head
        nc.sync.dma_start(
            qkb[:], qkvb_d[: 2 * D].rearrange("(o p) -> p o", p=HD)
        )

        ones128 = sb.tile([P, 1], F32R, tag="ones128")
        nc.sync.dma_start(ones128[:], ones_d[:, :1])
        ones1 = sb.tile([1, HD], F32R, tag="ones1")
        nc.sync.dma_start(ones1[:], ones_d[:1, :])

        qT = sb.tile([HD, H, S], F32R, tag="qT")
        kT = sb.tile([HD, H, S], F32R, tag="kT")
        v = sb.tile([P, 2, D], F32R, tag="v")
        oT = sb.tile([HD, H, S], F32R, tag="oT")
        out = sb.tile([P, 2, D], F32, tag="out")

        with tc.tile_pool(name="psA", bufs=3, space="PSUM") as psA:
            # ---- v in natural layout [tok, d]: lhsT = hT chunks ----
            VG = 384  # column group
            for g in range(3):
                wv = wvp.tile([P, ND, VG], F32R, tag="wv")
                nc.sync.dma_start(
                    wv[:],
                    qkvw_d.rearrange("(o p) c -> p o c", p=P)[
                        :, :, ds(2 * D + g * VG, VG)
                    ],
                )
                for tc_ in range(2):
                    ps = psA.tile([P, 512], F32, tag="psA", name="psA")[:, :VG]
                    for dc in range(ND):
                        nc.tensor.matmul(
                            ps,
                            hTt[:, dc, ts(tc_, P)],
                            wv[:, dc],
                            start=(dc == 0),
                            stop=(dc == ND - 1),
                        )
                    nc.vector.tensor_copy(v[:, tc_, ds(g * VG, VG)], ps)

            # ---- qT, kT: 4-head weight groups (DMA runs >= 512B) ----
            HG4 = 4
            for g in range(2):
                for hg in range(H // HG4):
                    wg = wgp.tile([P, ND, HG4 * HD], F32R, tag="wg")
                    nc.sync.dma_start(
                        wg[:],
                        qkvw_d.rearrange("(o p) c -> p o c", p=P)[
                            :, :, ds(g * D + hg * HG4 * HD, HG4 * HD)
                        ],
                    )
                    for i in range(HG4):
                        h = hg * HG4 + i
                        ps = psA.tile([P, 512], F32, tag="psA", name="psA")[
                            :HD, :S
                        ]
                        for dc in range(ND):
                            nc.tensor.matmul(
                                ps,
                                wg[:, dc, ds(i * HD, HD)],
                                hTt[:, dc],
                                start=(dc == 0),
                                stop=(dc == ND - 1),
                            )
                        dst = qT if g == 0 else kT
                        nc.scalar.activation(
                            dst[:, h],
                            ps,
                            IDENT,
                            bias=qkb[:, g * H + h : g * H + h + 1],
                        )

        # ---- attention per head (scoresT layout [key j, query q]) ----
        with (
            tc.tile_pool(name="psS", bufs=3, space="PSUM") as psS,
            tc.tile_pool(name="psX", bufs=4, space="PSUM") as psX,
        ):
            for h in range(H):
                expT = small.tile([P, 2, S], F32R, tag="expT")
                for jc in range(2):
                    ps = psS.tile([P, S], F32, tag="psS", name="psS")
                    nc.tensor.matmul(
                        ps,
                        kT[:, h, ts(jc, P)],
                        qT[:, h],
                        start=True,
                        stop=True,
                    )
                    nc.scalar.activation(expT[:, jc], ps, EXP, scale=scale)
                den = psX.tile([P, S], F32, tag="psX", name="psX")[:1]
                for jc in range(2):
                    nc.tensor.matmul(
                        den,
                        ones128,
                        expT[:, jc],
                        start=(jc == 0),
                        stop=(jc == 1),
                    )
                dinv = small.tile([1, S], F32R, tag="dinv")
                with nc.allow_low_precision(reason="f32r keeps full fp32 bits"):
                    nc.vector.reciprocal(dinv[:], den)
                ou = psX.tile([P, S], F32, tag="psX", name="psX")[:HD]
                for jc in range(2):
                    nc.tensor.matmul(
                        ou,
                        v[:, jc, ds(h * HD, HD)],
                        expT[:, jc],
                        start=(jc == 0),
                        stop=(jc == 1),
                    )
                bc = psX.tile([P, S], F32, tag="psX", name="psX")[:HD]
                nc.tensor.matmul(bc, ones1, dinv[:], start=True, stop=True)
                binv = small.tile([HD, S], F32, tag="binv")
                nc.scalar.copy(binv[:], bc)
                nc.vector.tensor_mul(oT[:, h], ou, binv[:])

        # ---- output projection, natural layout out[tok, col] ----
        with tc.tile_pool(name="psP", bufs=6, space="PSUM") as psP:
            PG = 384
            psums = [
                psP.tile([P, PG], F32, tag="psP", name=f"psP{i}") for i in range(6)
            ]
            for h in range(H):
                pw = pwp.tile([HD, D], F32R, tag="pw", name="pw")
                nc.sync.dma_start(pw[:], projw_d[ds(h * HD, HD), :])
                for tc_ in range(2):
                    for g in range(3):
                        nc.tensor.matmul(
                            psums[tc_ * 3 + g],
                            oT[:, h, ts(tc_, P)],
                            pw[:, ds(g * PG, PG)],
                            start=(h == 0),
                            stop=(h == H - 1),
                        )
            for tc_ in range(2):
                for g in range(3):
                    nc.vector.tensor_copy(
                        out[:, tc_, ds(g * PG, PG)], psums[tc_ * 3 + g]
                    )

        nc.sync.dma_start(
            attn_d.rearrange("(o p) c -> p o c", p=P), out[:]
        )

    nc.compile()
    return nc


# --------------------------------------------------------------------------
# Phase 2: two half-expert MLP jobs per core.
# per job j: tok [D, Sj] (transposed, padded), w1 [D, HH], w2 [HH, D],
#            b1 [HH], b2 [D]   ->   out [D, Sj]
# out = (gelu_tanh(tok^T w1 + b1) w2 + b2)^T
# --------------------------------------------------------------------------
def _split_tiles(Sj):
    # chunks of <=512, all >=256 where possible, roughly balanced (larger
    # moving dims hide LDWEIGHTS better)
    import math

    n = max(1, math.ceil(Sj / 512))
    base = Sj // n
    tiles = [base + (1 if i < Sj - base * n else 0) for i in range(n)]
    return tiles


def _build_phase2(slot_sizes):
    nc = _new_nc()
    ins = {}
    outs = {}
    for j, Sj in enumerate(slot_sizes):
        ins[f"tok{j}"] = nc.dram_tensor(
            f"tok{j}", [D, Sj], F32R, kind="ExternalInput"
        ).ap()
        ins[f"w1_{j}"] = nc.dram_tensor(
            f"w1_{j}", [D, HH], F32R, kind="ExternalInput"
        ).ap()
        ins[f"w2_{j}"] = nc.dram_tensor(
            f"w2_{j}", [HH, D], F32R, kind="ExternalInput"
        ).ap()
        ins[f"b1_{j}"] = nc.dram_tensor(
            f"b1_{j}", [HH], F32, kind="ExternalInput"
        ).ap()
        ins[f"b2_{j}"] = nc.dram_tensor(
            f"b2_{j}", [D], F32, kind="ExternalInput"
        ).ap()
        outs[f"out{j}"] = nc.dram_tensor(
            f"out{j}", [D, Sj], F32, kind="ExternalOutput"
        ).ap()

    with tile.TileContext(nc) as tc, ExitStack() as ctx:
        tokp = ctx.enter_context(tc.tile_pool(name="tok", bufs=1))
        w1p = ctx.enter_context(tc.tile_pool(name="w1", bufs=16))
        w2p = ctx.enter_context(tc.tile_pool(name="w2", bufs=31))
        h1p = ctx.enter_context(tc.tile_pool(name="h1", bufs=1))
        outp = ctx.enter_context(tc.tile_pool(name="out", bufs=3))
        bp = ctx.enter_context(tc.tile_pool(name="bias", bufs=1))
        pp = ctx.enter_context(tc.tile_pool(name="ps", bufs=8, space="PSUM"))

        maxS = max(slot_sizes)
        toks, b1s, b2s = [], [], []
        tiles_j, toffs_j = [], []
        for j, Sj in enumerate(slot_sizes):
            tiles = _split_tiles(Sj)
            tiles_j.append(tiles)
            toffs_j.append(np.concatenate([[0], np.cumsum(tiles)]).astype(int))
            tokT = tokp.tile([P, ND, Sj], F32R, tag=f"tok{j}", name=f"tok{j}")
            nc.sync.dma_start(
                tokT[:], ins[f"tok{j}"].rearrange("(o p) s -> p o s", p=P)
            )
            toks.append(tokT)
            b1t = bp.tile([P, NH], F32, tag=f"b1_{j}", name=f"b1_{j}")
            nc.sync.dma_start(b1t[:], ins[f"b1_{j}"].rearrange("(o p) -> p o", p=P))
            b1s.append(b1t)
            b2t = bp.tile([P, ND], F32, tag=f"b2_{j}", name=f"b2_{j}")
            nc.sync.dma_start(b2t[:], ins[f"b2_{j}"].rearrange("(o p) -> p o", p=P))
            b2s.append(b2t)

        # h1 buffer shared across jobs (job j+1's h1 writes follow job j's
        # eo reads, which is the PE-sequential order anyway)
        h1 = h1p.tile([P, NH, maxS], F32R, tag="h1", name="h1")

        HG = 4  # w1 column group (heads of residency)
        DG = 4  # w2 column group
        for j, Sj in enumerate(slot_sizes):
            tiles, toffs = tiles_j[j], toffs_j[j]
            nt = len(tiles)

            # ---- h1 = gelu(tok^T w1 + b1), chain-per-psum form ----
            for hg0 in range(0, NH, HG):
                hgn = min(HG, NH - hg0)
                w1ts = []
                for dc in range(ND):
                    w1t = w1p.tile([P, HG * P], F32R, tag="w1", name="w1t")[
                        :, : hgn * P
                    ]
                    nc.sync.dma_start(
                        w1t, ins[f"w1_{j}"][ds(dc * P, P), ds(hg0 * P, hgn * P)]
                    )
                    w1ts.append(w1t)
                for i in range(hgn):
                    hc = hg0 + i
                    for ti in range(nt):
                        ps = pp.tile([P, 512], F32, tag="ps", name="ps")[
                            :, : tiles[ti]
                        ]
                        for dc in range(ND):
                            nc.tensor.matmul(
                                ps,
                                w1ts[dc][:, ts(i, P)],
                                toks[j][:, dc, toffs[ti] : toffs[ti + 1]],
                                start=(dc == 0),
                                stop=(dc == ND - 1),
                            )
                        nc.scalar.activation(
                            h1[:, hc, toffs[ti] : toffs[ti + 1]],
                            ps,
                            GELU,
                            bias=b1s[j][:, hc : hc + 1],
                        )

            # ---- out = h1^T w2 + b2, bias applied in PSUM, DMA from PSUM ----
            out_r = outs[f"out{j}"].rearrange("(o p) s -> p o s", p=P)
            for dg0 in range(0, ND, DG):
                dgn = min(DG, ND - dg0)
                w2ts = []
                for hc in range(NH):
                    w2t = w2p.tile([P, DG * P], F32R, tag="w2", name="w2t")[
                        :, : dgn * P
                    ]
                    nc.sync.dma_start(
                        w2t, ins[f"w2_{j}"][ds(hc * P, P), ds(dg0 * P, dgn * P)]
                    )
                    w2ts.append(w2t)
                for i in range(dgn):
                    dc = dg0 + i
                    for ti in range(nt):
                        ps = pp.tile([P, 512], F32, tag="ps", name="ps")[
                            :, : tiles[ti]
                        ]
                        for hc in range(NH):
                            nc.tensor.matmul(
                                ps,
                                w2ts[hc][:, ts(i, P)],
                                h1[:, hc, toffs[ti] : toffs[ti + 1]],
                                start=(hc == 0),
                                stop=(hc == NH - 1),
                            )
                        ot = outp.tile([P, 512], F32, tag="ot", name="ot")[
                            :, : tiles[ti]
                        ]
                        nc.scalar.activation(
                            ot, ps, IDENT, bias=b2s[j][:, dc : dc + 1]
                        )
                        nc.sync.dma_start(
                            out_r[:, dc, toffs[ti] : toffs[ti + 1]], ot
                        )

    nc.compile()
    return nc


def _get_phase1():
    if "p1" not in _prog_cache:
        _prog_cache["p1"] = _build_phase1()
    return _prog_cache["p1"]


def _get_phase2(slot_sizes):
    key = ("p2", tuple(slot_sizes))
    if key not in _prog_cache:
        _prog_cache[key] = _build_phase2(slot_sizes)
    return _prog_cache[key]


# --------------------------------------------------------------------------
# Host glue
# --------------------------------------------------------------------------
def _ln(x):
    m = x.mean(-1, keepdims=True)
    v = x.var(-1, keepdims=True)
    return (x - m) / np.sqrt(v + EPS)


def _round_up(x, m):
    return ((x + m - 1) // m) * m


def kernel(
    x,
    c,
    qkv_w,
    qkv_b,
    proj_w,
    proj_b,
    adaln_w,
    adaln_b,
    gate_w,
    gate_b,
    w1,
    b1,
    w2,
    b2,
):
    x = np.asarray(x, dtype=np.float32)
    c = np.asarray(c, dtype=np.float32)
    qkv_w = np.asarray(qkv_w, dtype=np.float32)
    qkv_b = np.asarray(qkv_b, dtype=np.float32)
    proj_w = np.asarray(proj_w, dtype=np.float32)
    proj_b = np.asarray(proj_b, dtype=np.float32)
    adaln_w = np.asarray(adaln_w, dtype=np.float32)
    adaln_b = np.asarray(adaln_b, dtype=np.float32)
    gate_w = np.asarray(gate_w, dtype=np.float32)
    gate_b = np.asarray(gate_b, dtype=np.float32)
    w1 = np.asarray(w1, dtype=np.float32)
    b1 = np.asarray(b1, dtype=np.float32)
    w2 = np.asarray(w2, dtype=np.float32)
    b2 = np.asarray(b2, dtype=np.float32)

    # adaLN modulation (tiny: [8,1152]@[1152,4608])
    silu_c = c / (1.0 + np.exp(-c))
    mod = silu_c @ adaln_w + adaln_b
    shift_msa, scale_msa, gate_msa, gate_mlp = np.split(mod, 4, axis=1)

    h = _ln(x) * (1.0 + scale_msa[:, None, :]) + shift_msa[:, None, :]

    # ---- phase 1: attention on device ----
    nc1 = _get_phase1()
    in_maps = [
        {
            "hT": np.ascontiguousarray(h[b].T),
            "qkv_w": qkv_w,
            "qkv_b": qkv_b,
            "proj_w": proj_w,
            "ones": _ONES,
        }
        for b in range(B)
    ]
    r1 = _run(nc1, in_maps)
    attn = np.stack([r1[b]["attn"] for b in range(B)])  # [B, S, D]

    # folded biases: o = att@v + 1 b_v^T (rows of att sum to 1)
    const_bias = qkv_b[2 * D :] @ proj_w + proj_b  # [D]
    attn = attn + const_bias

    x2 = x + gate_msa[:, None, :] * attn
    nx = _ln(x2).reshape(-1, D)  # [N, D]

    # ---- routing (tiny) ----
    logits = nx @ gate_w + gate_b  # [N, E]
    order = np.argsort(-logits, axis=1, kind="stable")
    top_idx = order[:, :2]  # [N, 2]
    tv = np.take_along_axis(logits, top_idx, axis=1)
    tv = tv - tv.max(axis=1, keepdims=True)
    ev = np.exp(tv)
    top_score = ev / ev.sum(axis=1, keepdims=True)  # [N, 2]

    N = nx.shape[0]
    tok_idx = [np.nonzero((top_idx == e).any(axis=1))[0] for e in range(E)]
    tok_w = [
        top_score[tok_idx[e], (top_idx[tok_idx[e]] == e).argmax(axis=1)]
        for e in range(E)
    ]
    counts = np.array([len(t) for t in tok_idx])

    # sorted pairing: big expert's halves share cores with a small expert's
    order_e = np.argsort(-counts, kind="stable")
    bigs, smalls = order_e[:4], order_e[4:][::-1]
    S0 = max(256, int(counts[bigs].max()))
    S1 = max(256, int(counts[smalls].max()))

    nxT = np.ascontiguousarray(nx.T)  # [D, N]

    def tokT_padded(e, Sj):
        t = np.zeros((D, Sj), dtype=np.float32)
        idx = tok_idx[e]
        t[:, : len(idx)] = nxT[:, idx]
        return t

    in_maps2 = []
    for i in range(4):
        eb, es = int(bigs[i]), int(smalls[i])
        tb, tsm = tokT_padded(eb, S0), tokT_padded(es, S1)
        for half in range(2):
            cs = slice(half * HH, (half + 1) * HH)
            in_maps2.append(
                {
                    "tok0": tb,
                    "w1_0": np.ascontiguousarray(w1[eb][:, cs]),
                    "w2_0": np.ascontiguousarray(w2[eb][cs, :]),
                    "b1_0": np.ascontiguousarray(b1[eb][cs]),
                    "b2_0": b2[eb] if half == 0 else np.zeros(D, np.float32),
                    "tok1": tsm,
                    "w1_1": np.ascontiguousarray(w1[es][:, cs]),
                    "w2_1": np.ascontiguousarray(w2[es][cs, :]),
                    "b1_1": np.ascontiguousarray(b1[es][cs]),
                    "b2_1": b2[es] if half == 0 else np.zeros(D, np.float32),
                }
            )

    nc2 = _get_phase2((S0, S1))
    r2 = _run(nc2, in_maps2)

    # ---- combine ----
    moe = np.zeros((N, D), dtype=np.float32)
    for i in range(4):
        eb, es = int(bigs[i]), int(smalls[i])
        ob = r2[2 * i]["out0"] + r2[2 * i + 1]["out0"]  # [D, S0]
        osm = r2[2 * i]["out1"] + r2[2 * i + 1]["out1"]
        moe[tok_idx[eb]] += tok_w[eb][:, None] * ob[:, : counts[eb]].T
        moe[tok_idx[es]] += tok_w[es][:, None] * osm[:, : counts[es]].T

    y = x2 + gate_mlp[:, None, :] * moe.reshape(B, S, D)
    return y.astype(np.float32)
